# revision 1
# baseline (speedup 1.0000x reference)
"""Chamfer distance kernel for Trainium2 (8 NeuronCores).

Problem: xyz1, xyz2 [B=4, N=M=8192, 3] f32.
  d[b,n,m] = ||x1-x2||^2 ; outputs dist1/idx1 = min/argmin over m,
  dist2/idx2 = min/argmin over n.

Strategy per core (8 cores = 4 batches x 2 halves):
  core c handles batch b=c//2, half h=c%2.
  Pass A: rows = xyz1[b, h*4096:(h+1)*4096], cols = all xyz2[b]  -> dist1/idx1 half
  Pass B: rows = xyz2[b, h*4096:(h+1)*4096], cols = all xyz1[b]  -> dist2/idx2 half

On-chip, per 128-row block:
  - PE computes full distance tiles via one fused K=24 bf16-split matmul
    (see KDIM comment): psum[p, f] = d(row p, col f) to ~f32 accuracy
  - ACT copies PSUM -> SBUF X (source for the argmin gather)
  - DVE tensor_reduce (3D view) computes WIN-wide window minima TMF
  - v = min(TMF) is the distance output; winning window s* found via
    is_equal + (eq * descending-iota, max-reduce)
  - GPSIMD ap_gather fetches each partition's winning WIN-window (indices
    are shared per 16-partition group, so 15/16 of the gathered slots are
    other partitions' windows of the same row; the CG constant masks all
    foreign slots to zero, which also makes tie-breaking exactly
    "first index" like the reference)
  - final index = WIN*s* + r*; the G-chain is emitted one block late so
    the in-order DVE queue never stalls on the Pool gather
"""

import sys

sys.path.insert(0, "/opt/trn_rl_repo")

import numpy as np

import concourse.bacc as bacc
import concourse.mybir as mybir
from concourse.tile import TileContext
from concourse.bass_utils import run_bass_kernel_spmd

F32 = mybir.dt.float32
BF16 = mybir.dt.bfloat16
I32 = mybir.dt.int32
I16 = mybir.dt.int16
AX = mybir.AxisListType.X
OP = mybir.AluOpType

# bf16-split matmul: each f32 operand is split exactly into h+m+l bf16
# parts; per coordinate we keep the 6 largest cross products (hh, hm, mh,
# mm, hl, lh; the dropped ml/lm/ll are < 2^-24 relative).  sq1/sq2 ride in
# as 3 split rows each against exact 'ones'.  All products are exact in
# f32; only the PSUM f32 accumulation rounds, so the distance matches a
# straight f32 computation to ~1-2 ulp.
KDIM = 24
WIN = 16          # argmin gather window width
GENW = 2048       # psum generation width (4 banks)


def build_nc(nblk, m, n_cores=8, stages=3, repeat=1):
    """nblk: number of 128-row blocks per pass; m: rhs width (cols).

    stages (debug): 1 = matmul+reduce+v only, 2 = +F chain (no gather),
    3 = full.  repeat (debug): run the whole body N times (for slope
    timing)."""
    nrow = nblk * 128
    ngen = m // GENW
    nwin = m // WIN            # windows per block row
    wpg = GENW // WIN          # windows per generation

    nc = bacc.Bacc("TRN2", target_bir_lowering=False, debug=False,
                   num_devices=n_cores)

    la_d = nc.dram_tensor("la", [KDIM, nrow], BF16, kind="ExternalInput")
    ra_d = nc.dram_tensor("ra", [KDIM, m], BF16, kind="ExternalInput")
    lb_d = nc.dram_tensor("lb", [KDIM, nrow], BF16, kind="ExternalInput")
    rb_d = nc.dram_tensor("rb", [KDIM, m], BF16, kind="ExternalInput")
    cf_d = nc.dram_tensor("cf", [128, nwin], F32, kind="ExternalInput")
    cg_d = nc.dram_tensor("cg", [128, 16 * WIN], F32, kind="ExternalInput")

    da_d = nc.dram_tensor("da", [128, nblk], F32, kind="ExternalOutput")
    ia_d = nc.dram_tensor("ia", [128, nblk], I32, kind="ExternalOutput")
    db_d = nc.dram_tensor("db", [128, nblk], F32, kind="ExternalOutput")
    ib_d = nc.dram_tensor("ib", [128, nblk], I32, kind="ExternalOutput")

    with TileContext(nc) as tc:
        with (
            tc.tile_pool(name="const", bufs=1) as cpool,
            tc.tile_pool(name="psum", bufs=2, space="PSUM") as ppool,
            tc.tile_pool(name="x", bufs=3) as xpool,
            tc.tile_pool(name="tmf", bufs=4) as tmfpool,
            tc.tile_pool(name="scr", bufs=3) as spool,
            tc.tile_pool(name="small", bufs=8) as mpool,
            tc.tile_pool(name="acc", bufs=1) as apool,
        ):
            LA = cpool.tile([KDIM, nrow], BF16, tag="la")
            RA = cpool.tile([KDIM, m], BF16, tag="ra")
            LB = cpool.tile([KDIM, nrow], BF16, tag="lb")
            RB = cpool.tile([KDIM, m], BF16, tag="rb")
            CF = cpool.tile([128, nwin], F32, tag="cf")
            CG = cpool.tile([128, 16 * WIN], F32, tag="cg")
            nc.sync.dma_start(LA[:], la_d[:])
            nc.sync.dma_start(RA[:], ra_d[:])
            nc.sync.dma_start(LB[:], lb_d[:])
            nc.sync.dma_start(RB[:], rb_d[:])
            nc.sync.dma_start(CF[:], cf_d[:])
            nc.sync.dma_start(CG[:], cg_d[:])

            pass_cfgs = [
                (LA, RA, da_d, ia_d, "a"),
                (LB, RB, db_d, ib_d, "b"),
            ] * repeat
            def back_chain(st):
                # deferred argmin extraction (one block behind, so the DVE
                # queue never head-of-line blocks on the Pool gather)
                v, sstar, G, iacc, b = st
                eqG = spool.tile([128, 16 * WIN], F32, tag="eqg")
                nc.vector.tensor_scalar(eqG[:], G[:], v, None, op0=OP.is_equal)
                scrG = spool.tile([128, 16 * WIN], F32, tag="scrg")
                rd = mpool.tile([128, 1], F32, tag="rd")
                nc.vector.tensor_mul(scrG[:], eqG[:], CG[:])
                nc.vector.tensor_reduce(rd[:], scrG[:], axis=AX, op=OP.max)
                # idx = WIN*s* + r* = WIN*s* + (WIN - rd)
                t2 = mpool.tile([128, 1], F32, tag="t2")
                nc.vector.tensor_scalar(t2[:], rd[:], -1.0, float(WIN),
                                        op0=OP.mult, op1=OP.add)
                t3 = mpool.tile([128, 1], F32, tag="t3")
                nc.vector.tensor_scalar(t3[:], sstar[:], float(WIN), None,
                                        op0=OP.mult)
                nc.vector.tensor_add(iacc[:, b:b + 1], t3[:], t2[:])

            pending = []
            for rep in range(repeat):
              accs = {}
              for (L, R, d_out, i_out, acctag) in pass_cfgs[:2]:
                dacc_t = apool.tile([128, nblk], F32, tag=f"dacc{acctag}")
                iacc_t = apool.tile([128, nblk], F32, tag=f"iacc{acctag}")
                accs[acctag] = (dacc_t, iacc_t)
              # interleave the two passes block-by-block so every engine has
              # independent work in flight at block boundaries
              sched = [(cfg, b) for b in range(nblk) for cfg in pass_cfgs[:2]]
              for (L, R, d_out, i_out, acctag), b in sched:
                    dacc, iacc = accs[acctag]
                    X = xpool.tile([128, m], F32, tag="x")
                    TMF = tmfpool.tile([128, nwin], F32, tag="tmf")
                    lslice = L[:, b * 128:(b + 1) * 128]
                    for g in range(ngen):
                        ps = ppool.tile([128, GENW], F32, tag="ps")
                        for q in range(GENW // 512):
                            nc.tensor.matmul(
                                ps[:, q * 512:(q + 1) * 512],
                                lslice,
                                R[:, g * GENW + q * 512: g * GENW + (q + 1) * 512],
                                start=True, stop=True,
                            )
                        nc.scalar.copy(X[:, g * GENW:(g + 1) * GENW], ps[:])
                        if g == 0:
                            # split the first gen's reduce in half: PSUM deps
                            # are bank-level, so DVE starts after only 2 of 4
                            # matmuls -- trims the block-boundary stall
                            hw2 = GENW // 2
                            for h in range(2):
                                nc.vector.tensor_reduce(
                                    TMF[:, h * (wpg // 2):(h + 1) * (wpg // 2)],
                                    ps[:, h * hw2:(h + 1) * hw2].rearrange(
                                        "p (s r) -> p s r", r=WIN),
                                    axis=AX, op=OP.min,
                                )
                        else:
                            nc.vector.tensor_reduce(
                                TMF[:, g * wpg:(g + 1) * wpg],
                                ps.rearrange("p (s r) -> p s r", r=WIN),
                                axis=AX, op=OP.min,
                            )
                    # v (the min distance) straight into the output accumulator
                    v = dacc[:, b:b + 1]
                    nc.vector.tensor_reduce(v, TMF[:], axis=AX, op=OP.min)
                    if stages < 2:
                        nc.vector.tensor_copy(iacc[:, b:b + 1], v)
                        continue
                    # winning window s*: eqF = (TMF == v); sd = max(eqF * (nwin - s))
                    eqF = spool.tile([128, nwin], F32, tag="eqf")
                    nc.vector.tensor_scalar(eqF[:], TMF[:], v, None, op0=OP.is_equal)
                    scrF = spool.tile([128, nwin], F32, tag="scrf")
                    sd = mpool.tile([128, 1], F32, tag="sd")
                    nc.vector.tensor_mul(scrF[:], eqF[:], CF[:])
                    nc.vector.tensor_reduce(sd[:], scrF[:], axis=AX, op=OP.max)
                    # winning window index s* = nwin - sd
                    sstar = mpool.tile([128, 1], F32, tag="sstar")
                    nc.vector.tensor_scalar(sstar[:], sd[:], -1.0, float(nwin),
                                            op0=OP.mult, op1=OP.add)
                    if stages < 3:
                        nc.vector.tensor_copy(iacc[:, b:b + 1], sstar)
                        continue
                    gidx16 = mpool.tile([128, 1], I16, tag="gidx16")
                    nc.vector.tensor_copy(gidx16[:], sstar[:])
                    G = spool.tile([128, 16 * WIN], F32, tag="g")
                    nc.gpsimd.ap_gather(
                        G.rearrange("p (i r) -> p i r", r=WIN),
                        X.rearrange("p (s r) -> p s r", r=WIN),
                        gidx16[:],
                        channels=128, num_elems=nwin, d=WIN, num_idxs=16,
                    )
                    if len(pending) >= 2:
                        back_chain(pending.pop(0))
                    pending.append((v, sstar, G, iacc, b))
              while pending:
                  back_chain(pending.pop(0))
              for (L, R, d_out, i_out, acctag) in pass_cfgs[:2]:
                  dacc, iacc = accs[acctag]
                  ii = apool.tile([128, nblk], I32, tag=f"ii{acctag}")
                  nc.vector.tensor_copy(ii[:], iacc[:])
                  nc.sync.dma_start(d_out[:], dacc[:])
                  nc.sync.dma_start(i_out[:], ii[:])

    nc.compile()
    return nc


def _const_cf(nwin):
    # descending window iota: value nwin - s at window s, replicated rows
    return np.broadcast_to(
        (nwin - np.arange(nwin, dtype=np.float32)), (128, nwin)).copy()


def _const_cg():
    # (WIN - r) in each partition's own gather slot (j == p % 16), else 0.
    # Masking foreign slots makes tie-breaking exactly "first index" and
    # removes any junk-window contamination.
    out = np.zeros((128, 16 * WIN), dtype=np.float32)
    r = np.arange(WIN, dtype=np.float32)
    for p in range(128):
        j = p % 16
        out[p, j * WIN:(j + 1) * WIN] = WIN - r
    return out


import ml_dtypes

BF = ml_dtypes.bfloat16


def _split3(x):
    """Exact 3-way bf16 split: x ~= h + m + l (residual < 2^-24 rel)."""
    x = x.astype(np.float32)
    h = x.astype(BF)
    r = x - h.astype(np.float32)
    m = r.astype(BF)
    r2 = r - m.astype(np.float32)
    l = r2.astype(BF)
    return h, m, l


def _prep_l(pts):
    """pts [n,3] f32 -> lhsT [24, n] bf16 (see KDIM comment)."""
    n = pts.shape[0]
    out = np.empty((KDIM, n), dtype=BF)
    a = (-2.0 * pts.T).astype(np.float32)  # exact power-of-two scale
    for c in range(3):
        ah, am, al = _split3(a[c])
        out[6 * c + 0] = ah
        out[6 * c + 1] = ah
        out[6 * c + 2] = am
        out[6 * c + 3] = am
        out[6 * c + 4] = ah
        out[6 * c + 5] = al
    one = np.ones((n,), dtype=BF)
    out[18] = one
    out[19] = one
    out[20] = one
    s1h, s1m, s1l = _split3((pts * pts).sum(axis=1, dtype=np.float32))
    out[21] = s1h
    out[22] = s1m
    out[23] = s1l
    return out


def _prep_r(pts):
    """pts [m,3] f32 -> rhs [24, m] bf16 (see KDIM comment)."""
    mm = pts.shape[0]
    out = np.empty((KDIM, mm), dtype=BF)
    b = pts.T.astype(np.float32)
    for c in range(3):
        bh, bm, bl = _split3(b[c])
        out[6 * c + 0] = bh
        out[6 * c + 1] = bm
        out[6 * c + 2] = bh
        out[6 * c + 3] = bm
        out[6 * c + 4] = bl
        out[6 * c + 5] = bh
    s2h, s2m, s2l = _split3((pts * pts).sum(axis=1, dtype=np.float32))
    out[18] = s2h
    out[19] = s2m
    out[20] = s2l
    one = np.ones((mm,), dtype=BF)
    out[21] = one
    out[22] = one
    out[23] = one
    return out


_NC_CACHE = {}


def _get_nc(nblk, m):
    key = (nblk, m)
    if key not in _NC_CACHE:
        _NC_CACHE[key] = build_nc(nblk, m)
    return _NC_CACHE[key]


def kernel(xyz1, xyz2):
    xyz1 = np.asarray(xyz1, dtype=np.float32)
    xyz2 = np.asarray(xyz2, dtype=np.float32)
    B, N, _ = xyz1.shape
    M = xyz2.shape[1]
    assert (B, N, M) == (4, 8192, 8192), (B, N, M)
    half = N // 2
    nblk = half // 128

    nc = _get_nc(nblk, M)
    cf = _const_cf(M // WIN)
    cg = _const_cg()

    in_maps = []
    for c in range(8):
        b, h = divmod(c, 2)
        in_maps.append({
            "la": _prep_l(xyz1[b, h * half:(h + 1) * half]),
            "ra": _prep_r(xyz2[b]),
            "lb": _prep_l(xyz2[b, h * half:(h + 1) * half]),
            "rb": _prep_r(xyz1[b]),
            "cf": cf,
            "cg": cg,
        })

    res = run_bass_kernel_spmd(nc, in_maps, core_ids=list(range(8)))

    dist1 = np.empty((B, N), dtype=np.float32)
    idx1 = np.empty((B, N), dtype=np.int32)
    dist2 = np.empty((B, M), dtype=np.float32)
    idx2 = np.empty((B, M), dtype=np.int32)
    for c in range(8):
        b, h = divmod(c, 2)
        sl = slice(h * half, (h + 1) * half)
        r = res.results[c]
        dist1[b, sl] = r["da"].T.reshape(-1)
        idx1[b, sl] = r["ia"].T.reshape(-1)
        dist2[b, sl] = r["db"].T.reshape(-1)
        idx2[b, sl] = r["ib"].T.reshape(-1)
    return dist1, dist2, idx1, idx2



# revision 2
# speedup vs baseline: 1.7500x; 1.7500x over previous
"""Chamfer distance kernel for Trainium2 (8 NeuronCores).

Problem: xyz1, xyz2 [B=4, N=M=8192, 3] f32.
  d[b,n,m] = ||x1-x2||^2 ; outputs dist1/idx1 = min/argmin over m,
  dist2/idx2 = min/argmin over n.

Strategy per core (8 cores = 4 batches x 2 halves):
  core c handles batch b=c//2, half h=c%2.
  Pass A: rows = xyz1[b, h*4096:(h+1)*4096], cols = all xyz2[b]  -> idx1 windows
  Pass B: rows = xyz2[b, h*4096:(h+1)*4096], cols = all xyz1[b]  -> idx2 windows

Device computes, per row, the TOPK best 32-column windows of the NEGATED
partial distance e = 2*x.y - ||y||^2 (the per-row ||x||^2 term is constant
along the reduction axis, so it cannot change the argmin and is dropped).
The host then recomputes exact f32 distances over the TOPK*32 candidate
columns per row and takes the true min -> exact dist + idx outputs.

On-chip per 128-row block:
  - PE: fp8e5m2 DoubleRow matmuls (26 K-pairs = 51 split-product rows + pad).
    The e5m2 6-level split keeps |e - e_exact| ~1e-4; DR pair-sums round at
    ~11 bits which is why TOPK=4 windows are output (miss rate ~1e-4).
  - DVE: windowed max (W=32) straight from PSUM -> TMF [128, 256]
  - DVE: max8 + max_index on TMF -> top-8 window indices; first-index tie
    semantics matches the reference argmin.
  - top-TOPK window ids accumulate in a [128, TOPK*nblk] u16 tile, DMA'd out
    once per pass.
"""

import sys

sys.path.insert(0, "/opt/trn_rl_repo")

import numpy as np
import ml_dtypes

import concourse.bacc as bacc
import concourse.mybir as mybir
from concourse.tile import TileContext
from concourse.bass_utils import run_bass_kernel_spmd

F32 = mybir.dt.float32
F8E5 = mybir.dt.float8e5
U16 = mybir.dt.uint16
AX = mybir.AxisListType.X
OP = mybir.AluOpType
DR = mybir.MatmulPerfMode.DoubleRow

E5 = ml_dtypes.float8_e5m2

MAXSUM = 6                    # e5m2 split: keep product pairs with i+j <= MAXSUM
NSQ = 6                       # e5m2 parts of ||y||^2
PAIRS = [(i, j) for i in range(1, MAXSUM) for j in range(1, MAXSUM)
         if i + j <= MAXSUM]  # 15 pairs
NROWS = len(PAIRS) * 3 + NSQ  # 51
KP = (NROWS + 1) // 2         # 26 DoubleRow K-pairs (row 51 zero-padded)

W = 32                        # window width (columns per candidate window)
TOPK = 4                      # windows output per row
GENW = 2048                   # psum generation width (4 banks)


def build_nc(nblk, m, n_cores=8, repeat=1):
    """nblk: number of 128-row blocks per pass; m: rhs width (cols)."""
    nrow = nblk * 128
    ngen = m // GENW
    nwin = m // W              # windows per block row
    wpg = GENW // W            # windows per generation

    nc = bacc.Bacc("TRN2", target_bir_lowering=False, debug=False,
                   num_devices=n_cores)

    la_d = nc.dram_tensor("la", [KP, 2 * nrow], F8E5, kind="ExternalInput")
    ra_d = nc.dram_tensor("ra", [KP, 2 * m], F8E5, kind="ExternalInput")
    lb_d = nc.dram_tensor("lb", [KP, 2 * nrow], F8E5, kind="ExternalInput")
    rb_d = nc.dram_tensor("rb", [KP, 2 * m], F8E5, kind="ExternalInput")

    ia_d = nc.dram_tensor("ia", [128, TOPK * nblk], U16, kind="ExternalOutput")
    ib_d = nc.dram_tensor("ib", [128, TOPK * nblk], U16, kind="ExternalOutput")

    with TileContext(nc) as tc:
        with (
            tc.tile_pool(name="const", bufs=1) as cpool,
            tc.tile_pool(name="psum", bufs=2, space="PSUM") as ppool,
            tc.tile_pool(name="tmf", bufs=3) as tmfpool,
            tc.tile_pool(name="small", bufs=8) as mpool,
            tc.tile_pool(name="acc", bufs=1) as apool,
        ):
            LA = cpool.tile([KP, 2 * nrow], F8E5, tag="la")
            RA = cpool.tile([KP, 2 * m], F8E5, tag="ra")
            LB = cpool.tile([KP, 2 * nrow], F8E5, tag="lb")
            RB = cpool.tile([KP, 2 * m], F8E5, tag="rb")
            nc.sync.dma_start(LA[:], la_d[:])
            nc.sync.dma_start(RA[:], ra_d[:])
            nc.sync.dma_start(LB[:], lb_d[:])
            nc.sync.dma_start(RB[:], rb_d[:])

            for rep in range(repeat):
                pass_cfgs = [
                    (LA, RA, ia_d, "a"),
                    (LB, RB, ib_d, "b"),
                ]
                accs = {}
                for (_, _, _, acctag) in pass_cfgs:
                    iacc_t = apool.tile([128, TOPK * nblk], U16,
                                        tag=f"iacc{acctag}{rep}")
                    accs[acctag] = iacc_t
                sched = [(cfg, b) for b in range(nblk) for cfg in pass_cfgs]
                for (L, R, i_out, acctag), b in sched:
                    iacc = accs[acctag]
                    Lv = L.rearrange("k (two n) -> k two n", two=2)
                    Rv = R.rearrange("k (two n) -> k two n", two=2)
                    lslice = Lv[:, :, b * 128:(b + 1) * 128]
                    TMF = tmfpool.tile([128, nwin], F32, tag="tmf")
                    for g in range(ngen):
                        ps = ppool.tile([128, GENW], F32, tag="ps")
                        for q in range(GENW // 512):
                            c0 = g * GENW + q * 512
                            nc.tensor.matmul(
                                ps[:, q * 512:(q + 1) * 512],
                                lslice,
                                Rv[:, :, c0:c0 + 512],
                                start=True, stop=True,
                                perf_mode=DR,
                            )
                        nc.vector.tensor_reduce(
                            TMF[:, g * wpg:(g + 1) * wpg],
                            ps.rearrange("p (s r) -> p s r", r=W),
                            axis=AX, op=OP.max,
                        )
                    maxv = mpool.tile([128, 8], F32, tag="maxv")
                    nc.vector.max(maxv[:], TMF[:])
                    idx8 = mpool.tile([128, 8], U16, tag="idx8")
                    nc.vector.max_index(idx8[:], maxv[:], TMF[:])
                    nc.vector.tensor_copy(
                        iacc[:, b * TOPK:(b + 1) * TOPK], idx8[:, 0:TOPK])
                for (_, _, i_out, acctag) in pass_cfgs:
                    nc.sync.dma_start(i_out[:], accs[acctag][:])

    nc.compile()
    return nc


def _split_e5(x, n):
    parts, r = [], x.astype(np.float32)
    for _ in range(n):
        p = r.astype(E5)
        parts.append(p)
        r = r - p.astype(np.float32)
    return parts


def _pack_rows(rows, n):
    """rows: list of [n] arrays (f32 or e5m2) -> [KP, 2*n] e5m2 with row k of
    plane p at out[k, p*n:(p+1)*n]; pairing row r <-> slot (r%26? no: r<26 ->
    (r,0), else (r-26,1))."""
    out = np.zeros((KP, 2, n), dtype=E5)
    for r, row in enumerate(rows):
        k, pl = (r, 0) if r < KP else (r - KP, 1)
        out[k, pl] = row.astype(E5)
    return out.reshape(KP, 2 * n)


def _prep_l(pts):
    """pts [nr,3] f32 -> lhsT [KP, 2*nr] e5m2."""
    nr = pts.shape[0]
    a = (2.0 * pts).astype(np.float32)
    parts = [_split_e5(a[:, c], MAXSUM - 1) for c in range(3)]
    rows = []
    for (i, j) in PAIRS:
        for c in range(3):
            rows.append(parts[c][i - 1])
    one = np.ones((nr,), np.float32)
    for _ in range(NSQ):
        rows.append(one)
    return _pack_rows(rows, nr)


def _prep_r(pts):
    """pts [m,3] f32 -> rhs [KP, 2*m] e5m2."""
    mm = pts.shape[0]
    b = pts.astype(np.float32)
    parts = [_split_e5(b[:, c], MAXSUM - 1) for c in range(3)]
    rows = []
    for (i, j) in PAIRS:
        for c in range(3):
            rows.append(parts[c][j - 1])
    sq2 = (pts.astype(np.float32) ** 2).sum(axis=1, dtype=np.float32)
    for p in _split_e5(-sq2, NSQ):
        rows.append(p)
    return _pack_rows(rows, mm)


def _resolve(windows, x_rows, y_all, sq1_rows, sq2_all):
    """windows [nr, TOPK] u16; returns exact (dist [nr] f32, idx [nr] i32).

    Exact f32 distances over the TOPK*W candidate columns per row; argmin
    with first-index (smallest column) tie semantics like the reference.
    """
    nr = windows.shape[0]
    cols = (windows.astype(np.int64)[:, :, None] * W
            + np.arange(W)[None, None, :]).reshape(nr, TOPK * W)
    cols = np.sort(cols, axis=1)
    yc = y_all[cols]                          # [nr, TOPK*W, 3]
    prod = np.einsum('nd,nkd->nk', x_rows, yc, dtype=np.float32)
    d = (sq1_rows[:, None] + sq2_all[cols]).astype(np.float32) - \
        (2.0 * prod).astype(np.float32)
    d = d.astype(np.float32)
    am = np.argmin(d, axis=1)
    rr = np.arange(nr)
    return d[rr, am], cols[rr, am].astype(np.int32)


_NC_CACHE = {}


def _get_nc(nblk, m):
    key = (nblk, m)
    if key not in _NC_CACHE:
        _NC_CACHE[key] = build_nc(nblk, m)
    return _NC_CACHE[key]


def _win_from_acc(arr, nblk):
    """device iacc [128, TOPK*nblk] -> [nblk*128, TOPK] row-major windows."""
    return arr.reshape(128, nblk, TOPK).transpose(1, 0, 2).reshape(-1, TOPK)


def kernel(xyz1, xyz2):
    xyz1 = np.asarray(xyz1, dtype=np.float32)
    xyz2 = np.asarray(xyz2, dtype=np.float32)
    B, N, _ = xyz1.shape
    M = xyz2.shape[1]
    assert (B, N, M) == (4, 8192, 8192), (B, N, M)
    half = N // 2
    nblk = half // 128

    nc = _get_nc(nblk, M)

    in_maps = []
    for c in range(8):
        b, h = divmod(c, 2)
        in_maps.append({
            "la": _prep_l(xyz1[b, h * half:(h + 1) * half]),
            "ra": _prep_r(xyz2[b]),
            "lb": _prep_l(xyz2[b, h * half:(h + 1) * half]),
            "rb": _prep_r(xyz1[b]),
        })

    res = run_bass_kernel_spmd(nc, in_maps, core_ids=list(range(8)))

    dist1 = np.empty((B, N), dtype=np.float32)
    idx1 = np.empty((B, N), dtype=np.int32)
    dist2 = np.empty((B, M), dtype=np.float32)
    idx2 = np.empty((B, M), dtype=np.int32)
    sq1 = (xyz1 ** 2).sum(axis=2, dtype=np.float32)
    sq2 = (xyz2 ** 2).sum(axis=2, dtype=np.float32)
    for c in range(8):
        b, h = divmod(c, 2)
        sl = slice(h * half, (h + 1) * half)
        r = res.results[c]
        wa = _win_from_acc(r["ia"], nblk)
        wb = _win_from_acc(r["ib"], nblk)
        dist1[b, sl], idx1[b, sl] = _resolve(
            wa, xyz1[b, sl], xyz2[b], sq1[b, sl], sq2[b])
        dist2[b, sl], idx2[b, sl] = _resolve(
            wb, xyz2[b, sl], xyz1[b], sq2[b, sl], sq1[b])
    return dist1, dist2, idx1, idx2
